# revision 60
# baseline (speedup 1.0000x reference)
"""AdaptiveFeaturePropagation Trainium2 kernel (8 NeuronCores, SPMD).

Sharding: 8 cores = (batch 4) x (H halves 2); halos replicated host-side, no
collectives. Per core (shard rows [s, s+32)):
  conv1 (3x3, 1024->256, applied to cur & key): bf16 matmuls, contraction
    over Cin in 128-chunks, Winograd F(2,3) along W (4 m-planes), direct dy.
  conv2 (3x3, 512->256) in 8-row blocks and conv3 (1x1, 256->81).
  Spatially-variant 9x9 conv as a banded matmul with the contraction dim
  packed as (4 input rows x 32-col window) = 128:
    out[pix, c] = sum_{gi=0..2} band_gi[(ri,sx), pix].T @ ht4[q+gi][(ri,sx), c]
  over pixel blocks of 4 rows x {24,24,16} cols (24 blocks/core).  The band
  is built via a DRAM image round-trip: conv3 logits -> softmax numerators
  kt [96, 288] (col = 32*dy + dx, gaps zero) -> one 96-descriptor scatter
  (pixel stride 385, row stride 9248/6176, base 96) -> ONE 2D XBAR
  transpose-DMA read [[128, 12*wn], [1, 128]] per block (the strides make
  the (r, w, gi) composite mergeable; band col = 3*(wn*r+w) + gi).  All
  invalid (dy,dx) slots land in zeroed gaps of the image: no validity mask.
  Softmax denominators fold into the PSUM drain (per-partition scale).
  Queues: sync = band transposes + image zero-fills (+ startup keys/w1c0/
  ht4 j0-1), gpsimd = kt scatters, scalar = all bulk loads + outputs.
  A PE warm-up (16 self-matmuls on a zeroed tile) raises the tensor-engine
  p-state before the first conv matmul.
  Pipeline interleaves svc blocks between conv1/conv2 blocks (conv3(b)
  ~4 blocks ahead of svc(b)) so the PE never idles (p-state stays high) and
  the scatter->reload DMA latency is hidden.
Output is written pixel-major [2048, 1024] bf16 per core; host transposes
and upcasts.
"""

import sys

sys.path.insert(0, "/opt/trn_rl_repo")

import numpy as np
import ml_dtypes

import concourse.bacc as bacc
import concourse.mybir as mybir
from concourse.bass_types import AP
from concourse.tile import TileContext
from concourse.bass_utils import run_bass_kernel_spmd

F32 = mybir.dt.float32
BF16 = mybir.dt.bfloat16
BF = ml_dtypes.bfloat16
AF = mybir.ActivationFunctionType

# ---------------- configuration ----------------


class Cfg:
    B = 4
    HALVES = 2
    H = 64
    W = 64
    C_IN = 1024  # conv1 input channels
    CO1 = 256  # conv1 output channels (per branch)
    C2 = 512  # conv2 input channels
    CO2 = 256  # conv2 output channels
    K81 = 81
    CH = 1024  # high-feature channels

    OUT_ROWS = 32  # output rows per shard
    # conv1 output rows = OUT_ROWS + 2 (halo +-1), input rows = OUT_ROWS + 4
    C1_BLOCKS = ((0, 4), (4, 8), (12, 8), (20, 8), (28, 6))
    C2_ROWS = 8  # conv2 block rows (N = 512)

    # --- SVC band-image geometry ---
    # kt image: addr = PR*r + PW*w + S + 32*dy + dx; band read for group gi:
    # addr = S - dxoff + 128*gi + (32*ri + sx) + 383*(wn*r + w), one 2D
    # transpose-DMA per gi.  All invalid (dy, dx) slots land in zero gaps.
    PW = 385
    S = 96
    KTW = 288  # kt tile cols = 32*dy + dx (dx 0..8 valid, rest zero)
    XSZ = 37120  # per-block image elems (= 290*128)
    # (w0, wn, PR, sx0, dxoff) per w-group; PR - 32 = 384*wn so the whole
    # (r, w, gi) read composite merges to [[128, 3*4*wn], [1, 128]] - ONE
    # transpose-DMA per block, band col = 3*(wn*r + w) + gi
    WGROUPS = ((0, 24, 9248, 0, 0), (24, 24, 9248, 24, 0), (48, 16, 6176, 40, 8))
    N_KT = 8  # kt ring depth (gap cols stay zero across reuses)

    @property
    def X2_ROWS(self):
        return self.OUT_ROWS + 2

    @property
    def IN_ROWS(self):
        return self.OUT_ROWS + 4

    @property
    def HT_ROWS(self):
        return self.OUT_ROWS + 8

    @property
    def WP(self):
        return self.W + 2

    @property
    def WH(self):
        return self.W + 8

    @property
    def N_BLOCKS(self):
        return (self.OUT_ROWS // 4) * 3


CFG = Cfg()

# ---------------- graph builder ----------------


def build_graph(cfg):
    nc = bacc.Bacc(None, target_bir_lowering=False)
    W = cfg.W
    n_cin_ch = cfg.C_IN // 128
    n_c2_ch = cfg.C2 // 128
    n_co1_h = cfg.CO1 // 128
    n_co2_h = cfg.CO2 // 128
    CWP = n_cin_ch * cfg.WP  # 528: chunk*W row pitch
    NB = cfg.N_BLOCKS

    cur_e = nc.declare_dram_parameter(
        "cur", [128, cfg.IN_ROWS, CWP], BF16, isOutput=False
    )
    key_e = nc.declare_dram_parameter(
        "key", [128, cfg.IN_ROWS, CWP], BF16, isOutput=False
    )
    high_e = nc.declare_dram_parameter(
        "highT", [cfg.HT_ROWS, cfg.WH, cfg.CH], BF16, isOutput=False
    )
    # w1 holds Winograd-F(2,3)-transformed weights:
    # [128ci, chunk, dy(3)*m(4)*co] so per-chunk loads are contiguous
    w1_e = nc.declare_dram_parameter(
        "w1", [128, n_cin_ch, 12 * cfg.CO1], BF16, isOutput=False
    )
    # w2 holds Winograd-F(2,3)-transformed weights [128ci, chunk, dy*m*co]
    w2_e = nc.declare_dram_parameter(
        "w2", [128, n_c2_ch, 12 * cfg.CO2], BF16, isOutput=False
    )
    w3_e = nc.declare_dram_parameter(
        "w3", [128, cfg.CO2 // 128, cfg.K81], BF16, isOutput=False
    )
    b1_e = nc.declare_dram_parameter("b1", [128, n_co1_h], F32, isOutput=False)
    b2_e = nc.declare_dram_parameter("b2", [128, n_co2_h], F32, isOutput=False)
    b3_e = nc.declare_dram_parameter("b3", [128, cfg.K81], F32, isOutput=False)
    hmask_e = nc.declare_dram_parameter("hmask", [128, 2], F32, isOutput=False)
    out_e = nc.declare_dram_parameter(
        "out", [cfg.OUT_ROWS * W, cfg.CH], BF16, isOutput=True
    )

    ximgs = [nc.dram_tensor(f"ximg{b}", [cfg.XSZ], BF16) for b in range(NB)]

    with TileContext(nc) as tc:
        with (
            tc.tile_pool(name="const", bufs=1) as cpool,
            tc.tile_pool(name="feat", bufs=1) as fpool,
            tc.tile_pool(name="c1in", bufs=3) as inpool,
            tc.tile_pool(name="dw", bufs=3) as dpool,
            tc.tile_pool(name="wg", bufs=4) as wpool,
            tc.tile_pool(name="ht", bufs=15) as htpool,
            tc.tile_pool(name="band", bufs=8) as bandpool,
            tc.tile_pool(name="kt", bufs=cfg.N_KT) as ktpool,
            tc.tile_pool(name="small", bufs=8) as spool,
            tc.tile_pool(name="rd", bufs=16) as rdpool,
            tc.tile_pool(name="ob", bufs=5) as obpool,
            tc.tile_pool(name="ps", bufs=8, space="PSUM") as pspool,
        ):
            # ---- persistent constants ----
            w1sb = cpool.tile([128, n_cin_ch * 12 * cfg.CO1], BF16)
            b1sb = cpool.tile([128, n_co1_h], F32)
            hmsb = cpool.tile([128, 2], F32)
            w2sb = cpool.tile([128, n_c2_ch * 12 * cfg.CO2], BF16)
            w3sb = cpool.tile([128, (cfg.CO2 // 128) * cfg.K81], BF16)
            b2sb = cpool.tile([128, n_co2_h], F32)
            b3sb = cpool.tile([128, cfg.K81], F32)
            zt = cpool.tile([128, cfg.XSZ // 128], BF16)  # 289 cols

            W1CH = 12 * cfg.CO1  # per-chunk w1 stride in sbuf

            def load_w1_chunk(ch, eng=None):
                (eng or nc.scalar).dma_start(
                    out=w1sb[:, ch * W1CH : (ch + 1) * W1CH],
                    in_=w1_e[:, ch, :],
                )

            # PE p-state warm-up: ~16 self-matmuls on a zeroed tile so the
            # tensor engine is at full clock when real work arrives
            wutile = cpool.tile([128, 512], BF16)

            def emit_warmup():
                nc.vector.memset(wutile[:], 0.0)
                psw = pspool.tile([128, 512], F32, tag="ps", name="psw_wu")
                for i in range(16):
                    nc.tensor.matmul(
                        psw[:, :], wutile[:, 0:128], wutile[:, :],
                        start=True, stop=True, skip_group_check=True,
                    )

            # kt ring: gap cols are zeroed once; Exp rewrites only the 81
            # valid slots on reuse, so gaps stay zero
            kt_bufs = [
                ktpool.tile([96, cfg.KTW], BF16, tag="kt", name=f"ktb{i}")
                for i in range(cfg.N_KT)
            ]

            def emit_zero_fills():
                # zero-fill band images (gpsimd queue; deferred past the
                # startup DMA burst, needed before the first kt scatter)
                nc.vector.memset(zt[:], 0.0)
                for b in range(NB):
                    dz = AP(
                        ximgs[b],
                        0,
                        [[cfg.XSZ // 128, 128], [1, cfg.XSZ // 128]],
                    )
                    nc.sync.dma_start(out=dz, in_=zt[:])
                for k in kt_bufs:
                    nc.vector.memset(k[:], 0.0)

            def emit_deferred_consts():
                emit_zero_fills()
                nc.scalar.dma_start(out=w2sb[:], in_=w2_e[:, :, :])
                nc.scalar.dma_start(out=w3sb[:], in_=w3_e[:, :, :])
                nc.scalar.dma_start(out=b2sb[:], in_=b2_e[:, :])
                nc.scalar.dma_start(out=b3sb[:], in_=b3_e[:, :])

            # x2 (conv1 out, conv2 in), bf16, padded cols; x3 (conv2 out)
            # only the two never-written pad columns need zeroing
            x2c = []
            for i in range(2 * n_co1_h):
                t_ = fpool.tile([128, cfg.X2_ROWS * cfg.WP], BF16, tag=f"x2_{i}")
                v5 = t_[:, :].rearrange(
                    "p (r two w2) -> p r two w2", two=2, w2=cfg.WP // 2
                )
                nc.vector.memset(v5[:, :, 0, 0:1], 0.0)
                nc.vector.memset(v5[:, :, 1, cfg.WP // 2 - 1 : cfg.WP // 2], 0.0)
                x2c.append(t_)
            x3c = []
            for i in range(n_co2_h):
                t_ = fpool.tile([128, cfg.OUT_ROWS * W], BF16, tag=f"x3_{i}")
                x3c.append(t_)

            # highT tiles: ht4[(j, g)] = rows 4j..4j+3 x window g, partitions
            # = (ri-major, 32-col window) = 128, free = c (1024)
            ht4 = {}

            def need_ht4(j, eng=None):
                if j > cfg.HT_ROWS // 4 - 1 or (j, 0) in ht4:
                    return
                for g, (w0, wn, PR, sx0, dxoff) in enumerate(cfg.WGROUPS):
                    h_ = htpool.tile([128, cfg.CH], BF16, tag="ht4")
                    src = AP(
                        high_e,
                        (4 * j * cfg.WH + sx0) * cfg.CH,
                        [
                            [cfg.WH * cfg.CH, 4],
                            [cfg.CH, 32],
                            [1, cfg.CH],
                        ],
                    )
                    (eng or nc.scalar).dma_start(out=h_[:, :], in_=src)
                    ht4[(j, g)] = h_

            # ---- conv1 (cur, key) -> x2, Winograd F(2,3) along W ----
            # y[2j]   = m1 + m2 + m3,  y[2j+1] = m2 - m3 - m4 where
            # m_i = D_i . gw_i with D1 = d0-d2, D2 = d1+d2, D3 = d2-d1,
            # D4 = d1-d3 over padded cols (2j, 2j+1, 2j+2, 2j+3); vertical
            # taps stay direct (dy row shifts of the shared D planes).
            def emit_c1_block(bi, after_inputs=None):
                o0, nout = cfg.C1_BLOCKS[bi]
                nin = nout + 2
                J = W // 2
                # keys ride the sync queue only before the first transpose
                keng = nc.sync if bi < 2 else nc.scalar
                its = []
                for inp_e, eng in ((cur_e, nc.scalar), (key_e, keng)):
                    it = inpool.tile([128, nin * CWP], BF16, tag="c1in")
                    eng.dma_start(out=it[:], in_=inp_e[:, o0 : o0 + nin, :])
                    its.append(it)
                if after_inputs is not None:
                    after_inputs()
                for ii, it in enumerate(its):
                    itv5 = it[:, :].rearrange(
                        "p (r c two w2) -> p r c two w2",
                        c=n_cin_ch, two=2, w2=cfg.WP // 2,
                    )
                    psm = [
                        [
                            pspool.tile(
                                [128, nout * J], F32, tag="ps",
                                name=f"psw_{o0}_{ii}_{m_}_{h_}",
                            )
                            for h_ in range(n_co1_h)
                        ]
                        for m_ in range(4)
                    ]
                    for ch in range(n_cin_ch):
                        dt = dpool.tile([128, nin * 4 * J], BF16, tag="d")
                        dtv = dt[:, :].rearrange("p (r m j) -> p r m j", m=4, j=J)
                        s0 = itv5[:, :, ch, 0, 0:J]
                        s1 = itv5[:, :, ch, 1, 0:J]
                        s2 = itv5[:, :, ch, 0, 1 : J + 1]
                        s3 = itv5[:, :, ch, 1, 1 : J + 1]
                        nc.vector.tensor_sub(dtv[:, :, 0, :], s0, s2)
                        nc.vector.tensor_add(dtv[:, :, 1, :], s1, s2)
                        nc.vector.tensor_sub(dtv[:, :, 2, :], s2, s1)
                        nc.vector.tensor_sub(dtv[:, :, 3, :], s1, s3)
                        for dy in range(3):
                            for m_ in range(4):
                                rhs = dtv[:, dy : dy + nout, m_, :]
                                for hf in range(n_co1_h):
                                    c0 = ch * W1CH + (dy * 4 + m_) * cfg.CO1 + 128 * hf
                                    nc.tensor.matmul(
                                        psm[m_][hf][:, :],
                                        w1sb[:, c0 : c0 + 128],
                                        rhs,
                                        start=(ch == 0 and dy == 0),
                                        stop=(ch == n_cin_ch - 1 and dy == 2),
                                    )
                    for hf in range(n_co1_h):
                        p1, p2, p3, p4 = (psm[m_][hf][:, :] for m_ in range(4))
                        # TensorTensor reads at most one PSUM input: stage m2
                        t2 = wpool.tile([128, nout * J], F32, tag="w2c")
                        ta = wpool.tile([128, nout * J], F32, tag="wya")
                        tb = wpool.tile([128, nout * J], F32, tag="wyb")
                        nc.vector.tensor_copy(t2[:, :], p2)
                        nc.vector.tensor_add(ta[:, :], p1, t2[:, :])
                        nc.vector.tensor_add(ta[:, :], ta[:, :], p3)
                        nc.vector.tensor_sub(tb[:, :], t2[:, :], p3)
                        nc.vector.tensor_sub(tb[:, :], tb[:, :], p4)
                        x2v5 = x2c[ii * n_co1_h + hf][:, :].rearrange(
                            "p (r two w2) -> p r two w2", two=2, w2=cfg.WP // 2
                        )
                        dst_even = x2v5[:, o0 : o0 + nout, 1, 0:J]
                        dst_odd = x2v5[:, o0 : o0 + nout, 0, 1 : J + 1]
                        nc.scalar.activation(
                            dst_even, ta[:, :], AF.Relu, bias=b1sb[:, hf : hf + 1]
                        )
                        nc.scalar.activation(
                            dst_odd, tb[:, :], AF.Relu, bias=b1sb[:, hf : hf + 1]
                        )

            # halo row masking (rows 0 and X2_ROWS-1 of x2)
            lr = cfg.X2_ROWS - 1

            def emit_mask_top():
                for i in range(2 * n_co1_h):
                    nc.vector.tensor_scalar_mul(
                        x2c[i][:, 0 : cfg.WP], x2c[i][:, 0 : cfg.WP], hmsb[:, 0:1]
                    )

            def emit_mask_bot():
                for i in range(2 * n_co1_h):
                    nc.vector.tensor_scalar_mul(
                        x2c[i][:, lr * cfg.WP : (lr + 1) * cfg.WP],
                        x2c[i][:, lr * cfg.WP : (lr + 1) * cfg.WP],
                        hmsb[:, 1:2],
                    )

            # ---- conv2 -> x3 (row blocks), Winograd F(2,3) along W ----
            W2CH = 12 * cfg.CO2

            def emit_c2_block(r0, nr):
                nin2 = nr + 2
                J = W // 2
                psm = [
                    [
                        pspool.tile(
                            [128, nr * J], F32, tag="ps",
                            name=f"ps2w_{r0}_{m_}_{h_}",
                        )
                        for h_ in range(n_co2_h)
                    ]
                    for m_ in range(4)
                ]
                for ch in range(n_c2_ch):
                    x2v = x2c[ch][:, :].rearrange(
                        "p (r two w2) -> p r two w2", two=2, w2=cfg.WP // 2
                    )
                    dt = dpool.tile([128, nin2 * 4 * J], BF16, tag="d")
                    dtv = dt[:, :].rearrange("p (r m j) -> p r m j", m=4, j=J)
                    s0 = x2v[:, r0 : r0 + nin2, 0, 0:J]
                    s1 = x2v[:, r0 : r0 + nin2, 1, 0:J]
                    s2 = x2v[:, r0 : r0 + nin2, 0, 1 : J + 1]
                    s3 = x2v[:, r0 : r0 + nin2, 1, 1 : J + 1]
                    nc.vector.tensor_sub(dtv[:, :, 0, :], s0, s2)
                    nc.vector.tensor_add(dtv[:, :, 1, :], s1, s2)
                    nc.vector.tensor_sub(dtv[:, :, 2, :], s2, s1)
                    nc.vector.tensor_sub(dtv[:, :, 3, :], s1, s3)
                    for dy in range(3):
                        for m_ in range(4):
                            rhs = dtv[:, dy : dy + nr, m_, :]
                            for hf in range(n_co2_h):
                                c0 = ch * W2CH + (dy * 4 + m_) * cfg.CO2 + 128 * hf
                                nc.tensor.matmul(
                                    psm[m_][hf][:, :],
                                    w2sb[:, c0 : c0 + 128],
                                    rhs,
                                    start=(ch == 0 and dy == 0),
                                    stop=(ch == n_c2_ch - 1 and dy == 2),
                                )
                for hf in range(n_co2_h):
                    p1, p2, p3, p4 = (psm[m_][hf][:, :] for m_ in range(4))
                    t2 = wpool.tile([128, nr * J], F32, tag="w2c")
                    ta = wpool.tile([128, nr * J], F32, tag="wya")
                    tb = wpool.tile([128, nr * J], F32, tag="wyb")
                    nc.vector.tensor_copy(t2[:, :], p2)
                    nc.vector.tensor_add(ta[:, :], p1, t2[:, :])
                    nc.vector.tensor_add(ta[:, :], ta[:, :], p3)
                    nc.vector.tensor_sub(tb[:, :], t2[:, :], p3)
                    nc.vector.tensor_sub(tb[:, :], tb[:, :], p4)
                    x3v = x3c[hf][:, :].rearrange(
                        "p (r w2 two) -> p r w2 two", w2=J, two=2
                    )
                    nc.scalar.activation(
                        x3v[:, r0 : r0 + nr, :, 0], ta[:, :],
                        AF.Relu, bias=b2sb[:, hf : hf + 1],
                    )
                    nc.scalar.activation(
                        x3v[:, r0 : r0 + nr, :, 1], tb[:, :],
                        AF.Relu, bias=b2sb[:, hf : hf + 1],
                    )

            # ---- per block (3 per row-quad): conv3 + softmax + band ----
            chains = {}
            kt_ctr = [0]

            def emit_conv3(b):
                q, g = divmod(b, 3)
                w0, wn, PR, sx0, dxoff = cfg.WGROUPS[g]
                M = 4 * wn
                if g == 0:
                    need_ht4(q + 2)  # prefetch the last tile svc(q) needs
                ps3 = pspool.tile([M, cfg.K81], F32, tag="ps")
                # stationary operand allows only one free dim: stage the
                # (4 x wn) pixel block contiguously first
                xst = spool.tile([128, 2 * 96], BF16, tag="xst")
                for ch in range(cfg.CO2 // 128):
                    x3v = x3c[ch][:, :].rearrange("p (r w) -> p r w", w=W)
                    nc.vector.tensor_copy(
                        xst[:, ch * 96 : ch * 96 + M],
                        x3v[:, 4 * q : 4 * q + 4, w0 : w0 + wn],
                    )
                for ch in range(cfg.CO2 // 128):
                    nc.tensor.matmul(
                        ps3[:, :],
                        xst[:, ch * 96 : ch * 96 + M],
                        w3sb[:, ch * cfg.K81 : (ch + 1) * cfg.K81],
                        start=(ch == 0),
                        stop=(ch == cfg.CO2 // 128 - 1),
                    )
                t81 = spool.tile([M, cfg.K81], F32, tag="t81")
                nc.vector.tensor_add(t81[:], ps3[:, :], b3sb[0:M, :])
                nc.scalar.activation(t81[:], t81[:], AF.Relu)
                kt = kt_bufs[kt_ctr[0] % cfg.N_KT]
                kt_ctr[0] += 1
                dsum = spool.tile([M, 1], F32, tag="dsum")
                ktv = kt[0:M, :].rearrange("p (dy c) -> p dy c", c=32)
                t81v = t81[:, :].rearrange("p (dy dx) -> p dy dx", dx=9)
                nc.scalar.activation(
                    ktv[:, :, 0:9], t81v, AF.Exp, accum_out=dsum[:]
                )
                rd = rdpool.tile([M, 1], F32, tag="rd")
                nc.vector.reciprocal(rd[:], dsum[:])
                # scatter kt (96 descriptors of 576B), then reload the three
                # band tiles as 2D XBAR transpose-DMAs
                dstap = AP(ximgs[b], cfg.S, [[PR, 4], [cfg.PW, wn], [1, cfg.KTW]])
                nc.gpsimd.dma_start(out=dstap, in_=kt[0:M, :])
                # NOTE: XBAR transpose is only reliable on the sync queue
                bt = bandpool.tile([128, 3 * 96], BF16, tag="band")
                srcap = AP(
                    ximgs[b], cfg.S - dxoff, [[128, 3 * M], [1, 128]]
                )
                nc.sync.dma_start(out=bt[:, 0 : 3 * M], in_=srcap, transpose=True)
                chains[b] = (bt, rd)

            def emit_svc(b, keep=0):
                bt, rd = chains.pop(b)
                q, g = divmod(b, 3)
                w0, wn, PR, sx0, dxoff = cfg.WGROUPS[g]
                M = 4 * wn
                bv = bt[:, 0 : 3 * M].rearrange("p (f g) -> p f g", g=3)
                ob = obpool.tile([96, 2 * 512], BF16, tag="ob")
                pvs = [
                    pspool.tile([M, 512], F32, tag="ps", name=f"pv_{b}_{i}")
                    for i in range(2)
                ]
                # p-state keepers: dependency-free self-matmuls run while the
                # band DMA is in flight; the real gi0 start=True re-zeroes
                for i in range(keep):
                    nc.tensor.matmul(
                        pvs[0][:, :], wutile[:, 0:M], wutile[:, :],
                        start=True, stop=True, skip_group_check=True,
                    )
                for cc in range(2):
                    pv = pvs[cc]
                    for gi in range(3):
                        nc.tensor.matmul(
                            pv[:, :],
                            bv[:, :, gi],
                            ht4[(q + gi, g)][:, 512 * cc : 512 * (cc + 1)],
                            start=(gi == 0),
                            stop=(gi == 2),
                        )
                    if cc == 0:
                        nc.scalar.activation(
                            ob[0:M, 0:512], pv[:, :], AF.Copy, scale=rd[:, 0:1]
                        )
                    else:
                        nc.vector.tensor_scalar_mul(
                            ob[0:M, 512:1024], pv[:, :], rd[:, 0:1]
                        )
                dst = AP(
                    out_e,
                    (4 * q * W + w0) * cfg.CH,
                    [[W * cfg.CH, 4], [cfg.CH, wn], [1, cfg.CH]],
                )
                nc.scalar.dma_start(out=dst, in_=ob[0:M, :])

            # ---- interleaved block-granularity pipeline ----
            def g3(b):
                emit_conv3(b)

            def svc(b, keep=0):
                emit_svc(b, keep)

            emit_warmup()
            load_w1_chunk(0, nc.sync)
            nc.scalar.dma_start(out=b1sb[:], in_=b1_e[:, :])
            nc.scalar.dma_start(out=hmsb[:], in_=hmask_e[:, :])
            emit_c1_block(
                0,
                after_inputs=lambda: [load_w1_chunk(c) for c in range(1, 8)],
            )
            emit_mask_top()
            emit_c1_block(1)
            emit_deferred_consts()
            need_ht4(0, nc.sync)
            need_ht4(1, nc.sync)
            emit_c2_block(0, 8)
            g3(0); g3(1); g3(2); g3(3)
            emit_c1_block(2)
            g3(4); g3(5); svc(0); svc(1)
            emit_c2_block(8, 8)
            g3(6); svc(2); g3(7); svc(3)
            emit_c1_block(3)
            g3(8); svc(4); g3(9); svc(5)
            emit_c2_block(16, 8)
            g3(10); svc(6); g3(11); svc(7)
            emit_c1_block(4)
            emit_mask_bot()
            g3(12); svc(8, keep=6); g3(13); svc(9)
            g3(14); svc(10); g3(15); svc(11)
            g3(16); svc(12); g3(17); svc(13)
            emit_c2_block(24, 8)
            g3(18); svc(14, keep=4); g3(19); svc(15, keep=4)
            g3(20); svc(16, keep=4); g3(21); svc(17, keep=4)
            g3(22); svc(18, keep=4); g3(23); svc(19, keep=4)
            svc(20, keep=6); svc(21, keep=6); svc(22, keep=6); svc(23, keep=6)

    return nc


# ---------------- host side ----------------

_CACHED = None


def _get_graph():
    global _CACHED
    if _CACHED is None:
        _CACHED = build_graph(CFG)
        _CACHED.compile()
    return _CACHED


def shard_inputs(inputs, cfg):
    """Build per-core input maps from the full problem inputs."""
    cur = np.asarray(inputs["current_frame_low_features"])
    key = np.asarray(inputs["key_frame_low_features"])
    high = np.asarray(inputs["key_frame_high_features"])
    B, Cin, H, W = cur.shape

    w_reduce = np.asarray(inputs["w_reduce"])  # (CO1, Cin, 3, 3)
    w2 = np.asarray(inputs["w2"])  # (CO2, C2, 3, 3)
    w3 = np.asarray(inputs["w3"])  # (81, CO2, 1, 1)
    n_cin_ch = Cin // 128
    n_c2_ch = cfg.C2 // 128
    # w1 host layout [128ci, chunk, dy*m*co], Winograd-F(2,3) transformed
    G = np.array(
        [[1, 0, 0], [0.5, 0.5, 0.5], [0.5, -0.5, 0.5], [0, 0, 1]], np.float32
    )
    wr = w_reduce.reshape(cfg.CO1, n_cin_ch, 128, 3, 3)  # o c p y d
    w1h = np.ascontiguousarray(
        np.einsum("md,ocpyd->pcymo", G, wr).reshape(128, n_cin_ch, 12 * cfg.CO1)
    ).astype(BF)
    wr2 = w2.reshape(cfg.CO2, n_c2_ch, 128, 3, 3)  # o c p y d
    w2h = np.ascontiguousarray(
        np.einsum("md,ocpyd->pcymo", G, wr2).reshape(128, n_c2_ch, 12 * cfg.CO2)
    ).astype(BF)
    w3h = np.ascontiguousarray(
        w3.reshape(cfg.K81, cfg.CO2 // 128, 128).transpose(2, 1, 0)
    ).astype(BF)
    b1h = np.ascontiguousarray(
        np.asarray(inputs["b_reduce"]).reshape(cfg.CO1 // 128, 128).T
    ).astype(np.float32)
    b2h = np.ascontiguousarray(
        np.asarray(inputs["b2"]).reshape(cfg.CO2 // 128, 128).T
    ).astype(np.float32)
    b3h = np.broadcast_to(
        np.asarray(inputs["b3"]).astype(np.float32)[None, :], (128, cfg.K81)
    ).copy()

    in_maps = []
    for core in range(B * cfg.HALVES):
        b, half = core // cfg.HALVES, core % cfg.HALVES
        s = half * cfg.OUT_ROWS
        # low features: rows [s-2, s+OUT_ROWS+2), w padded +-1, bf16,
        # layout [128, IN_ROWS, chunk*WP]
        lowpad = np.zeros((2, Cin, cfg.IN_ROWS, cfg.WP), np.float32)
        r0, r1 = s - 2, s + cfg.OUT_ROWS + 2
        cr0, cr1 = max(r0, 0), min(r1, H)
        lowpad[0, :, cr0 - r0 : cr1 - r0, 1 : 1 + W] = cur[b, :, cr0:cr1, :]
        lowpad[1, :, cr0 - r0 : cr1 - r0, 1 : 1 + W] = key[b, :, cr0:cr1, :]
        lowT = np.ascontiguousarray(
            lowpad.reshape(2, n_cin_ch, 128, cfg.IN_ROWS, cfg.WP // 2, 2)
            .transpose(0, 2, 3, 1, 5, 4)
        ).reshape(2, 128, cfg.IN_ROWS, n_cin_ch * cfg.WP).astype(BF)
        # high features: rows [s-4, s+OUT_ROWS+4), w padded +-4, transposed
        hp = np.zeros((cfg.HT_ROWS, cfg.WH, cfg.CH), np.float32)
        hr0, hr1 = s - 4, s + cfg.OUT_ROWS + 4
        chr0, chr1 = max(hr0, 0), min(hr1, H)
        hp[chr0 - hr0 : chr1 - hr0, 4 : 4 + W, :] = high[b, :, chr0:chr1, :].transpose(
            1, 2, 0
        )
        hmask = np.zeros((128, 2), np.float32)
        hmask[:, 0] = 0.0 if s == 0 else 1.0
        hmask[:, 1] = 0.0 if s + cfg.OUT_ROWS == H else 1.0
        in_maps.append(
            {
                "cur": lowT[0],
                "key": lowT[1],
                "highT": hp.astype(BF),
                "w1": w1h,
                "w2": w2h,
                "w3": w3h,
                "b1": b1h,
                "b2": b2h,
                "b3": b3h,
                "hmask": hmask,
            }
        )
    return in_maps


def gather_outputs(results, cfg, H, W):
    out = np.zeros((cfg.B, cfg.CH, H, W), np.float32)
    for core, res in enumerate(results):
        b, half = core // cfg.HALVES, core % cfg.HALVES
        s = half * cfg.OUT_ROWS
        o = np.asarray(res["out"]).astype(np.float32).reshape(
            cfg.OUT_ROWS, W, cfg.CH
        )
        out[b, :, s : s + cfg.OUT_ROWS, :] = o.transpose(2, 0, 1)
    return out


def kernel(**inputs) -> np.ndarray:
    cfg = CFG
    nc = _get_graph()
    in_maps = shard_inputs(inputs, cfg)
    res = run_bass_kernel_spmd(nc, in_maps, core_ids=list(range(8)))
    return gather_outputs(res.results, cfg, cfg.H, cfg.W)


# revision 61
# speedup vs baseline: 1.0437x; 1.0437x over previous
"""AdaptiveFeaturePropagation Trainium2 kernel (8 NeuronCores, SPMD).

Sharding: 8 cores = (batch 4) x (H halves 2); halos replicated host-side, no
collectives. Per core (shard rows [s, s+32)):
  conv1 (3x3, 1024->256, applied to cur & key): bf16 matmuls, contraction
    over Cin in 128-chunks, Winograd F(2,3) along W (4 m-planes), direct dy.
  conv2 (3x3, 512->256) in 8-row blocks and conv3 (1x1, 256->81).
  Spatially-variant 9x9 conv as a banded matmul with the contraction dim
  packed as (4 input rows x 32-col window) = 128:
    out[pix, c] = sum_{gi=0..2} band_gi[(ri,sx), pix].T @ ht4[q+gi][(ri,sx), c]
  over pixel blocks of 4 rows x {24,24,16} cols (24 blocks/core).  The band
  is built via a DRAM image round-trip: conv3 logits -> softmax numerators
  kt [96, 288] (col = 32*dy + dx, gaps zero) -> one 96-descriptor scatter
  (pixel stride 385, row stride 9248/6176, base 96) -> ONE 2D XBAR
  transpose-DMA read [[128, 12*wn], [1, 128]] per block (the strides make
  the (r, w, gi) composite mergeable; band col = 3*(wn*r+w) + gi).  All
  invalid (dy,dx) slots land in zeroed gaps of the image: no validity mask.
  Softmax denominators fold into the PSUM drain (per-partition scale).
  Queues: sync = band transposes + image zero-fills (+ startup keys/w1c0/
  ht4 j0-1), gpsimd = kt scatters, scalar = all bulk loads + outputs.
  A PE warm-up (16 self-matmuls on a zeroed tile) raises the tensor-engine
  p-state before the first conv matmul.
  Pipeline interleaves svc blocks between conv1/conv2 blocks (conv3(b)
  ~4 blocks ahead of svc(b)) so the PE never idles (p-state stays high) and
  the scatter->reload DMA latency is hidden.
Output is written pixel-major [2048, 1024] bf16 per core; host transposes
and upcasts.
"""

import sys

sys.path.insert(0, "/opt/trn_rl_repo")

import numpy as np
import ml_dtypes

import concourse.bacc as bacc
import concourse.mybir as mybir
from concourse.bass_types import AP
from concourse.tile import TileContext
from concourse.bass_utils import run_bass_kernel_spmd

F32 = mybir.dt.float32
BF16 = mybir.dt.bfloat16
BF = ml_dtypes.bfloat16
AF = mybir.ActivationFunctionType

# ---------------- configuration ----------------


class Cfg:
    B = 4
    HALVES = 2
    H = 64
    W = 64
    C_IN = 1024  # conv1 input channels
    CO1 = 256  # conv1 output channels (per branch)
    C2 = 512  # conv2 input channels
    CO2 = 256  # conv2 output channels
    K81 = 81
    CH = 1024  # high-feature channels

    OUT_ROWS = 32  # output rows per shard
    # conv1 output rows = OUT_ROWS + 2 (halo +-1), input rows = OUT_ROWS + 4
    C1_BLOCKS = ((0, 4), (4, 8), (12, 8), (20, 8), (28, 6))
    C2_ROWS = 8  # conv2 block rows (N = 512)

    # --- SVC band-image geometry ---
    # kt image: addr = PR*r + PW*w + S + 32*dy + dx; band read for group gi:
    # addr = S - dxoff + 128*gi + (32*ri + sx) + 383*(wn*r + w), one 2D
    # transpose-DMA per gi.  All invalid (dy, dx) slots land in zero gaps.
    PW = 385
    S = 96
    KTW = 288  # kt tile cols = 32*dy + dx (dx 0..8 valid, rest zero)
    XSZ = 37120  # per-block image elems (= 290*128)
    # (w0, wn, PR, sx0, dxoff) per w-group; PR - 32 = 384*wn so the whole
    # (r, w, gi) read composite merges to [[128, 3*4*wn], [1, 128]] - ONE
    # transpose-DMA per block, band col = 3*(wn*r + w) + gi
    WGROUPS = ((0, 24, 9248, 0, 0), (24, 24, 9248, 24, 0), (48, 16, 6176, 40, 8))
    N_KT = 8  # kt ring depth (gap cols stay zero across reuses)

    @property
    def X2_ROWS(self):
        return self.OUT_ROWS + 2

    @property
    def IN_ROWS(self):
        return self.OUT_ROWS + 4

    @property
    def HT_ROWS(self):
        return self.OUT_ROWS + 8

    @property
    def WP(self):
        return self.W + 2

    @property
    def WH(self):
        return self.W + 8

    @property
    def N_BLOCKS(self):
        return (self.OUT_ROWS // 4) * 3


CFG = Cfg()

# ---------------- graph builder ----------------


def build_graph(cfg):
    nc = bacc.Bacc(None, target_bir_lowering=False)
    W = cfg.W
    n_cin_ch = cfg.C_IN // 128
    n_c2_ch = cfg.C2 // 128
    n_co1_h = cfg.CO1 // 128
    n_co2_h = cfg.CO2 // 128
    CWP = n_cin_ch * cfg.WP  # 528: chunk*W row pitch
    NB = cfg.N_BLOCKS

    cur_e = nc.declare_dram_parameter(
        "cur", [128, cfg.IN_ROWS, CWP], BF16, isOutput=False
    )
    key_e = nc.declare_dram_parameter(
        "key", [128, cfg.IN_ROWS, CWP], BF16, isOutput=False
    )
    high_e = nc.declare_dram_parameter(
        "highT", [cfg.HT_ROWS, cfg.WH, cfg.CH], BF16, isOutput=False
    )
    # w1 holds Winograd-F(2,3)-transformed weights:
    # [128ci, chunk, dy(3)*m(4)*co] so per-chunk loads are contiguous
    w1_e = nc.declare_dram_parameter(
        "w1", [128, n_cin_ch, 12 * cfg.CO1], BF16, isOutput=False
    )
    # w2 holds Winograd-F(2,3)-transformed weights [128ci, chunk, dy*m*co]
    w2_e = nc.declare_dram_parameter(
        "w2", [128, n_c2_ch, 12 * cfg.CO2], BF16, isOutput=False
    )
    w3_e = nc.declare_dram_parameter(
        "w3", [128, cfg.CO2 // 128, cfg.K81], BF16, isOutput=False
    )
    b1_e = nc.declare_dram_parameter("b1", [128, n_co1_h], F32, isOutput=False)
    b2_e = nc.declare_dram_parameter("b2", [128, n_co2_h], F32, isOutput=False)
    b3_e = nc.declare_dram_parameter("b3", [128, cfg.K81], F32, isOutput=False)
    hmask_e = nc.declare_dram_parameter("hmask", [128, 2], F32, isOutput=False)
    out_e = nc.declare_dram_parameter(
        "out", [cfg.OUT_ROWS * W, cfg.CH], BF16, isOutput=True
    )

    ximgs = [nc.dram_tensor(f"ximg{b}", [cfg.XSZ], BF16) for b in range(NB)]

    with TileContext(nc) as tc:
        with (
            tc.tile_pool(name="const", bufs=1) as cpool,
            tc.tile_pool(name="feat", bufs=1) as fpool,
            tc.tile_pool(name="c1in", bufs=3) as inpool,
            tc.tile_pool(name="dw", bufs=3) as dpool,
            tc.tile_pool(name="wg", bufs=4) as wpool,
            tc.tile_pool(name="ht", bufs=15) as htpool,
            tc.tile_pool(name="band", bufs=8) as bandpool,
            tc.tile_pool(name="kt", bufs=cfg.N_KT) as ktpool,
            tc.tile_pool(name="small", bufs=8) as spool,
            tc.tile_pool(name="rd", bufs=16) as rdpool,
            tc.tile_pool(name="ob", bufs=5) as obpool,
            tc.tile_pool(name="ps", bufs=8, space="PSUM") as pspool,
        ):
            # ---- persistent constants ----
            w1sb = cpool.tile([128, n_cin_ch * 12 * cfg.CO1], BF16)
            b1sb = cpool.tile([128, n_co1_h], F32)
            hmsb = cpool.tile([128, 2], F32)
            w2sb = cpool.tile([128, n_c2_ch * 12 * cfg.CO2], BF16)
            w3sb = cpool.tile([128, (cfg.CO2 // 128) * cfg.K81], BF16)
            b2sb = cpool.tile([128, n_co2_h], F32)
            b3sb = cpool.tile([128, cfg.K81], F32)
            zt = cpool.tile([128, cfg.XSZ // 128], BF16)  # 289 cols

            W1CH = 12 * cfg.CO1  # per-chunk w1 stride in sbuf

            def load_w1_chunk(ch, eng=None):
                (eng or nc.scalar).dma_start(
                    out=w1sb[:, ch * W1CH : (ch + 1) * W1CH],
                    in_=w1_e[:, ch, :],
                )

            # PE p-state warm-up: ~16 self-matmuls on a zeroed tile so the
            # tensor engine is at full clock when real work arrives
            wutile = cpool.tile([128, 512], BF16)

            def emit_warmup():
                nc.vector.memset(wutile[:], 0.0)
                psw = pspool.tile([128, 512], F32, tag="ps", name="psw_wu")
                for i in range(16):
                    nc.tensor.matmul(
                        psw[:, :], wutile[:, 0:128], wutile[:, :],
                        start=True, stop=True, skip_group_check=True,
                    )

            # kt ring: gap cols are zeroed once; Exp rewrites only the 81
            # valid slots on reuse, so gaps stay zero
            kt_bufs = [
                ktpool.tile([96, cfg.KTW], BF16, tag="kt", name=f"ktb{i}")
                for i in range(cfg.N_KT)
            ]

            def emit_zero_fills():
                # zero-fill band images (gpsimd queue; deferred past the
                # startup DMA burst, needed before the first kt scatter)
                nc.vector.memset(zt[:], 0.0)
                for b in range(NB):
                    dz = AP(
                        ximgs[b],
                        0,
                        [[cfg.XSZ // 128, 128], [1, cfg.XSZ // 128]],
                    )
                    nc.sync.dma_start(out=dz, in_=zt[:])
                for k in kt_bufs:
                    nc.vector.memset(k[:], 0.0)

            def emit_deferred_consts():
                emit_zero_fills()
                nc.scalar.dma_start(out=w2sb[:], in_=w2_e[:, :, :])
                nc.scalar.dma_start(out=w3sb[:], in_=w3_e[:, :, :])
                nc.scalar.dma_start(out=b2sb[:], in_=b2_e[:, :])
                nc.scalar.dma_start(out=b3sb[:], in_=b3_e[:, :])

            # x2 (conv1 out, conv2 in), bf16, padded cols; x3 (conv2 out)
            # only the two never-written pad columns need zeroing
            x2c = []
            for i in range(2 * n_co1_h):
                t_ = fpool.tile([128, cfg.X2_ROWS * cfg.WP], BF16, tag=f"x2_{i}")
                v5 = t_[:, :].rearrange(
                    "p (r two w2) -> p r two w2", two=2, w2=cfg.WP // 2
                )
                nc.vector.memset(v5[:, :, 0, 0:1], 0.0)
                nc.vector.memset(v5[:, :, 1, cfg.WP // 2 - 1 : cfg.WP // 2], 0.0)
                x2c.append(t_)
            x3c = []
            for i in range(n_co2_h):
                t_ = fpool.tile([128, cfg.OUT_ROWS * W], BF16, tag=f"x3_{i}")
                x3c.append(t_)

            # highT tiles: ht4[(j, g)] = rows 4j..4j+3 x window g, partitions
            # = (ri-major, 32-col window) = 128, free = c (1024)
            ht4 = {}

            def need_ht4(j, eng=None):
                if j > cfg.HT_ROWS // 4 - 1 or (j, 0) in ht4:
                    return
                for g, (w0, wn, PR, sx0, dxoff) in enumerate(cfg.WGROUPS):
                    h_ = htpool.tile([128, cfg.CH], BF16, tag="ht4")
                    src = AP(
                        high_e,
                        (4 * j * cfg.WH + sx0) * cfg.CH,
                        [
                            [cfg.WH * cfg.CH, 4],
                            [cfg.CH, 32],
                            [1, cfg.CH],
                        ],
                    )
                    (eng or nc.scalar).dma_start(out=h_[:, :], in_=src)
                    ht4[(j, g)] = h_

            # ---- conv1 (cur, key) -> x2, Winograd F(2,3) along W ----
            # y[2j]   = m1 + m2 + m3,  y[2j+1] = m2 - m3 - m4 where
            # m_i = D_i . gw_i with D1 = d0-d2, D2 = d1+d2, D3 = d2-d1,
            # D4 = d1-d3 over padded cols (2j, 2j+1, 2j+2, 2j+3); vertical
            # taps stay direct (dy row shifts of the shared D planes).
            def emit_c1_block(bi, after_inputs=None):
                o0, nout = cfg.C1_BLOCKS[bi]
                nin = nout + 2
                J = W // 2
                # keys ride the sync queue only before the first transpose
                keng = nc.sync if bi < 2 else nc.scalar
                its = []
                for inp_e, eng in ((cur_e, nc.scalar), (key_e, keng)):
                    it = inpool.tile([128, nin * CWP], BF16, tag="c1in")
                    eng.dma_start(out=it[:], in_=inp_e[:, o0 : o0 + nin, :])
                    its.append(it)
                if after_inputs is not None:
                    after_inputs()
                for ii, it in enumerate(its):
                    itv5 = it[:, :].rearrange(
                        "p (r c two w2) -> p r c two w2",
                        c=n_cin_ch, two=2, w2=cfg.WP // 2,
                    )
                    psm = [
                        [
                            pspool.tile(
                                [128, nout * J], F32, tag="ps",
                                name=f"psw_{o0}_{ii}_{m_}_{h_}",
                            )
                            for h_ in range(n_co1_h)
                        ]
                        for m_ in range(4)
                    ]
                    for ch in range(n_cin_ch):
                        dt = dpool.tile([128, nin * 4 * J], BF16, tag="d")
                        dtv = dt[:, :].rearrange("p (r m j) -> p r m j", m=4, j=J)
                        s0 = itv5[:, :, ch, 0, 0:J]
                        s1 = itv5[:, :, ch, 1, 0:J]
                        s2 = itv5[:, :, ch, 0, 1 : J + 1]
                        s3 = itv5[:, :, ch, 1, 1 : J + 1]
                        nc.vector.tensor_sub(dtv[:, :, 0, :], s0, s2)
                        nc.vector.tensor_add(dtv[:, :, 1, :], s1, s2)
                        nc.vector.tensor_sub(dtv[:, :, 2, :], s2, s1)
                        nc.vector.tensor_sub(dtv[:, :, 3, :], s1, s3)
                        for dy in range(3):
                            for m_ in range(4):
                                rhs = dtv[:, dy : dy + nout, m_, :]
                                for hf in range(n_co1_h):
                                    c0 = ch * W1CH + (dy * 4 + m_) * cfg.CO1 + 128 * hf
                                    nc.tensor.matmul(
                                        psm[m_][hf][:, :],
                                        w1sb[:, c0 : c0 + 128],
                                        rhs,
                                        start=(ch == 0 and dy == 0),
                                        stop=(ch == n_cin_ch - 1 and dy == 2),
                                    )
                    for hf in range(n_co1_h):
                        p1, p2, p3, p4 = (psm[m_][hf][:, :] for m_ in range(4))
                        # TensorTensor reads at most one PSUM input: stage m2
                        t2 = wpool.tile([128, nout * J], F32, tag="w2c")
                        ta = wpool.tile([128, nout * J], F32, tag="wya")
                        tb = wpool.tile([128, nout * J], F32, tag="wyb")
                        nc.vector.tensor_copy(t2[:, :], p2)
                        nc.vector.tensor_add(ta[:, :], p1, t2[:, :])
                        nc.vector.tensor_add(ta[:, :], ta[:, :], p3)
                        nc.vector.tensor_sub(tb[:, :], t2[:, :], p3)
                        nc.vector.tensor_sub(tb[:, :], tb[:, :], p4)
                        x2v5 = x2c[ii * n_co1_h + hf][:, :].rearrange(
                            "p (r two w2) -> p r two w2", two=2, w2=cfg.WP // 2
                        )
                        dst_even = x2v5[:, o0 : o0 + nout, 1, 0:J]
                        dst_odd = x2v5[:, o0 : o0 + nout, 0, 1 : J + 1]
                        nc.scalar.activation(
                            dst_even, ta[:, :], AF.Relu, bias=b1sb[:, hf : hf + 1]
                        )
                        nc.scalar.activation(
                            dst_odd, tb[:, :], AF.Relu, bias=b1sb[:, hf : hf + 1]
                        )

            # halo row masking (rows 0 and X2_ROWS-1 of x2)
            lr = cfg.X2_ROWS - 1

            def emit_mask_top():
                for i in range(2 * n_co1_h):
                    nc.vector.tensor_scalar_mul(
                        x2c[i][:, 0 : cfg.WP], x2c[i][:, 0 : cfg.WP], hmsb[:, 0:1]
                    )

            def emit_mask_bot():
                for i in range(2 * n_co1_h):
                    nc.vector.tensor_scalar_mul(
                        x2c[i][:, lr * cfg.WP : (lr + 1) * cfg.WP],
                        x2c[i][:, lr * cfg.WP : (lr + 1) * cfg.WP],
                        hmsb[:, 1:2],
                    )

            # ---- conv2 -> x3 (row blocks), Winograd F(2,3) along W ----
            W2CH = 12 * cfg.CO2

            def emit_c2_block(r0, nr):
                nin2 = nr + 2
                J = W // 2
                psm = [
                    [
                        pspool.tile(
                            [128, nr * J], F32, tag="ps",
                            name=f"ps2w_{r0}_{m_}_{h_}",
                        )
                        for h_ in range(n_co2_h)
                    ]
                    for m_ in range(4)
                ]
                for ch in range(n_c2_ch):
                    x2v = x2c[ch][:, :].rearrange(
                        "p (r two w2) -> p r two w2", two=2, w2=cfg.WP // 2
                    )
                    dt = dpool.tile([128, nin2 * 4 * J], BF16, tag="d")
                    dtv = dt[:, :].rearrange("p (r m j) -> p r m j", m=4, j=J)
                    s0 = x2v[:, r0 : r0 + nin2, 0, 0:J]
                    s1 = x2v[:, r0 : r0 + nin2, 1, 0:J]
                    s2 = x2v[:, r0 : r0 + nin2, 0, 1 : J + 1]
                    s3 = x2v[:, r0 : r0 + nin2, 1, 1 : J + 1]
                    nc.vector.tensor_sub(dtv[:, :, 0, :], s0, s2)
                    nc.vector.tensor_add(dtv[:, :, 1, :], s1, s2)
                    nc.vector.tensor_sub(dtv[:, :, 2, :], s2, s1)
                    nc.vector.tensor_sub(dtv[:, :, 3, :], s1, s3)
                    for dy in range(3):
                        for m_ in range(4):
                            rhs = dtv[:, dy : dy + nr, m_, :]
                            for hf in range(n_co2_h):
                                c0 = ch * W2CH + (dy * 4 + m_) * cfg.CO2 + 128 * hf
                                nc.tensor.matmul(
                                    psm[m_][hf][:, :],
                                    w2sb[:, c0 : c0 + 128],
                                    rhs,
                                    start=(ch == 0 and dy == 0),
                                    stop=(ch == n_c2_ch - 1 and dy == 2),
                                )
                for hf in range(n_co2_h):
                    p1, p2, p3, p4 = (psm[m_][hf][:, :] for m_ in range(4))
                    t2 = wpool.tile([128, nr * J], F32, tag="w2c")
                    ta = wpool.tile([128, nr * J], F32, tag="wya")
                    tb = wpool.tile([128, nr * J], F32, tag="wyb")
                    nc.vector.tensor_copy(t2[:, :], p2)
                    nc.vector.tensor_add(ta[:, :], p1, t2[:, :])
                    nc.vector.tensor_add(ta[:, :], ta[:, :], p3)
                    nc.vector.tensor_sub(tb[:, :], t2[:, :], p3)
                    nc.vector.tensor_sub(tb[:, :], tb[:, :], p4)
                    x3v = x3c[hf][:, :].rearrange(
                        "p (r w2 two) -> p r w2 two", w2=J, two=2
                    )
                    nc.scalar.activation(
                        x3v[:, r0 : r0 + nr, :, 0], ta[:, :],
                        AF.Relu, bias=b2sb[:, hf : hf + 1],
                    )
                    nc.scalar.activation(
                        x3v[:, r0 : r0 + nr, :, 1], tb[:, :],
                        AF.Relu, bias=b2sb[:, hf : hf + 1],
                    )

            # ---- per block (3 per row-quad): conv3 + softmax + band ----
            chains = {}
            kt_ctr = [0]

            def emit_conv3(b):
                q, g = divmod(b, 3)
                w0, wn, PR, sx0, dxoff = cfg.WGROUPS[g]
                M = 4 * wn
                if g == 0:
                    need_ht4(q + 2)  # prefetch the last tile svc(q) needs
                ps3 = pspool.tile([M, cfg.K81], F32, tag="ps")
                # stationary operand allows only one free dim: stage the
                # (4 x wn) pixel block contiguously first
                xst = spool.tile([128, 2 * 96], BF16, tag="xst")
                for ch in range(cfg.CO2 // 128):
                    x3v = x3c[ch][:, :].rearrange("p (r w) -> p r w", w=W)
                    nc.vector.tensor_copy(
                        xst[:, ch * 96 : ch * 96 + M],
                        x3v[:, 4 * q : 4 * q + 4, w0 : w0 + wn],
                    )
                for ch in range(cfg.CO2 // 128):
                    nc.tensor.matmul(
                        ps3[:, :],
                        xst[:, ch * 96 : ch * 96 + M],
                        w3sb[:, ch * cfg.K81 : (ch + 1) * cfg.K81],
                        start=(ch == 0),
                        stop=(ch == cfg.CO2 // 128 - 1),
                    )
                t81 = spool.tile([M, cfg.K81], F32, tag="t81")
                nc.vector.tensor_add(t81[:], ps3[:, :], b3sb[0:M, :])
                nc.scalar.activation(t81[:], t81[:], AF.Relu)
                kt = kt_bufs[kt_ctr[0] % cfg.N_KT]
                kt_ctr[0] += 1
                dsum = spool.tile([M, 1], F32, tag="dsum")
                ktv = kt[0:M, :].rearrange("p (dy c) -> p dy c", c=32)
                t81v = t81[:, :].rearrange("p (dy dx) -> p dy dx", dx=9)
                nc.scalar.activation(
                    ktv[:, :, 0:9], t81v, AF.Exp, accum_out=dsum[:]
                )
                rd = rdpool.tile([M, 1], F32, tag="rd")
                nc.vector.reciprocal(rd[:], dsum[:])
                # scatter kt (96 descriptors of 576B), then reload the three
                # band tiles as 2D XBAR transpose-DMAs
                dstap = AP(ximgs[b], cfg.S, [[PR, 4], [cfg.PW, wn], [1, cfg.KTW]])
                nc.gpsimd.dma_start(out=dstap, in_=kt[0:M, :])
                # NOTE: XBAR transpose is only reliable on the sync queue
                bt = bandpool.tile([128, 3 * 96], BF16, tag="band")
                srcap = AP(
                    ximgs[b], cfg.S - dxoff, [[128, 3 * M], [1, 128]]
                )
                nc.sync.dma_start(out=bt[:, 0 : 3 * M], in_=srcap, transpose=True)
                chains[b] = (bt, rd)

            def emit_svc(b):
                bt, rd = chains.pop(b)
                q, g = divmod(b, 3)
                w0, wn, PR, sx0, dxoff = cfg.WGROUPS[g]
                M = 4 * wn
                bv = bt[:, 0 : 3 * M].rearrange("p (f g) -> p f g", g=3)
                ob = obpool.tile([96, 2 * 512], BF16, tag="ob")
                for cc in range(2):
                    pv = pspool.tile([M, 512], F32, tag="ps")
                    for gi in range(3):
                        nc.tensor.matmul(
                            pv[:, :],
                            bv[:, :, gi],
                            ht4[(q + gi, g)][:, 512 * cc : 512 * (cc + 1)],
                            start=(gi == 0),
                            stop=(gi == 2),
                        )
                    if cc == 0:
                        nc.scalar.activation(
                            ob[0:M, 0:512], pv[:, :], AF.Copy, scale=rd[:, 0:1]
                        )
                    else:
                        nc.vector.tensor_scalar_mul(
                            ob[0:M, 512:1024], pv[:, :], rd[:, 0:1]
                        )
                dst = AP(
                    out_e,
                    (4 * q * W + w0) * cfg.CH,
                    [[W * cfg.CH, 4], [cfg.CH, wn], [1, cfg.CH]],
                )
                nc.scalar.dma_start(out=dst, in_=ob[0:M, :])

            # ---- interleaved block-granularity pipeline ----
            def g3(b):
                emit_conv3(b)

            def svc(b):
                emit_svc(b)

            emit_warmup()
            load_w1_chunk(0, nc.sync)
            nc.scalar.dma_start(out=b1sb[:], in_=b1_e[:, :])
            nc.scalar.dma_start(out=hmsb[:], in_=hmask_e[:, :])
            emit_c1_block(
                0,
                after_inputs=lambda: [load_w1_chunk(c) for c in range(1, 8)],
            )
            emit_mask_top()
            emit_c1_block(1)
            emit_deferred_consts()
            need_ht4(0, nc.sync)
            need_ht4(1, nc.sync)
            emit_c2_block(0, 8)
            g3(0); g3(1); g3(2); g3(3)
            emit_c1_block(2)
            g3(4); g3(5); svc(0); svc(1)
            emit_c2_block(8, 8)
            g3(6); svc(2); g3(7); svc(3)
            emit_c1_block(3)
            g3(8); svc(4); g3(9); svc(5)
            emit_c2_block(16, 8)
            g3(10); svc(6); g3(11); svc(7)
            emit_c1_block(4)
            emit_mask_bot()
            g3(12); svc(8); g3(13); svc(9)
            g3(14); svc(10); g3(15); svc(11)
            g3(16); svc(12); g3(17); svc(13)
            emit_c2_block(24, 8)
            g3(18); svc(14); g3(19); svc(15)
            g3(20); svc(16); g3(21); svc(17)
            g3(22); svc(18); g3(23); svc(19)
            svc(20); svc(21); svc(22); svc(23)

    return nc


# ---------------- host side ----------------

_CACHED = None


def _get_graph():
    global _CACHED
    if _CACHED is None:
        _CACHED = build_graph(CFG)
        _CACHED.compile()
    return _CACHED


def shard_inputs(inputs, cfg):
    """Build per-core input maps from the full problem inputs."""
    cur = np.asarray(inputs["current_frame_low_features"])
    key = np.asarray(inputs["key_frame_low_features"])
    high = np.asarray(inputs["key_frame_high_features"])
    B, Cin, H, W = cur.shape

    w_reduce = np.asarray(inputs["w_reduce"])  # (CO1, Cin, 3, 3)
    w2 = np.asarray(inputs["w2"])  # (CO2, C2, 3, 3)
    w3 = np.asarray(inputs["w3"])  # (81, CO2, 1, 1)
    n_cin_ch = Cin // 128
    n_c2_ch = cfg.C2 // 128
    # w1 host layout [128ci, chunk, dy*m*co], Winograd-F(2,3) transformed
    G = np.array(
        [[1, 0, 0], [0.5, 0.5, 0.5], [0.5, -0.5, 0.5], [0, 0, 1]], np.float32
    )
    wr = w_reduce.reshape(cfg.CO1, n_cin_ch, 128, 3, 3)  # o c p y d
    w1h = np.ascontiguousarray(
        np.einsum("md,ocpyd->pcymo", G, wr).reshape(128, n_cin_ch, 12 * cfg.CO1)
    ).astype(BF)
    wr2 = w2.reshape(cfg.CO2, n_c2_ch, 128, 3, 3)  # o c p y d
    w2h = np.ascontiguousarray(
        np.einsum("md,ocpyd->pcymo", G, wr2).reshape(128, n_c2_ch, 12 * cfg.CO2)
    ).astype(BF)
    w3h = np.ascontiguousarray(
        w3.reshape(cfg.K81, cfg.CO2 // 128, 128).transpose(2, 1, 0)
    ).astype(BF)
    b1h = np.ascontiguousarray(
        np.asarray(inputs["b_reduce"]).reshape(cfg.CO1 // 128, 128).T
    ).astype(np.float32)
    b2h = np.ascontiguousarray(
        np.asarray(inputs["b2"]).reshape(cfg.CO2 // 128, 128).T
    ).astype(np.float32)
    b3h = np.broadcast_to(
        np.asarray(inputs["b3"]).astype(np.float32)[None, :], (128, cfg.K81)
    ).copy()

    in_maps = []
    for core in range(B * cfg.HALVES):
        b, half = core // cfg.HALVES, core % cfg.HALVES
        s = half * cfg.OUT_ROWS
        # low features: rows [s-2, s+OUT_ROWS+2), w padded +-1, bf16,
        # layout [128, IN_ROWS, chunk*WP]
        lowpad = np.zeros((2, Cin, cfg.IN_ROWS, cfg.WP), np.float32)
        r0, r1 = s - 2, s + cfg.OUT_ROWS + 2
        cr0, cr1 = max(r0, 0), min(r1, H)
        lowpad[0, :, cr0 - r0 : cr1 - r0, 1 : 1 + W] = cur[b, :, cr0:cr1, :]
        lowpad[1, :, cr0 - r0 : cr1 - r0, 1 : 1 + W] = key[b, :, cr0:cr1, :]
        lowT = np.ascontiguousarray(
            lowpad.reshape(2, n_cin_ch, 128, cfg.IN_ROWS, cfg.WP // 2, 2)
            .transpose(0, 2, 3, 1, 5, 4)
        ).reshape(2, 128, cfg.IN_ROWS, n_cin_ch * cfg.WP).astype(BF)
        # high features: rows [s-4, s+OUT_ROWS+4), w padded +-4, transposed
        hp = np.zeros((cfg.HT_ROWS, cfg.WH, cfg.CH), np.float32)
        hr0, hr1 = s - 4, s + cfg.OUT_ROWS + 4
        chr0, chr1 = max(hr0, 0), min(hr1, H)
        hp[chr0 - hr0 : chr1 - hr0, 4 : 4 + W, :] = high[b, :, chr0:chr1, :].transpose(
            1, 2, 0
        )
        hmask = np.zeros((128, 2), np.float32)
        hmask[:, 0] = 0.0 if s == 0 else 1.0
        hmask[:, 1] = 0.0 if s + cfg.OUT_ROWS == H else 1.0
        in_maps.append(
            {
                "cur": lowT[0],
                "key": lowT[1],
                "highT": hp.astype(BF),
                "w1": w1h,
                "w2": w2h,
                "w3": w3h,
                "b1": b1h,
                "b2": b2h,
                "b3": b3h,
                "hmask": hmask,
            }
        )
    return in_maps


def gather_outputs(results, cfg, H, W):
    out = np.zeros((cfg.B, cfg.CH, H, W), np.float32)
    for core, res in enumerate(results):
        b, half = core // cfg.HALVES, core % cfg.HALVES
        s = half * cfg.OUT_ROWS
        o = np.asarray(res["out"]).astype(np.float32).reshape(
            cfg.OUT_ROWS, W, cfg.CH
        )
        out[b, :, s : s + cfg.OUT_ROWS, :] = o.transpose(2, 0, 1)
    return out


def kernel(**inputs) -> np.ndarray:
    cfg = CFG
    nc = _get_graph()
    in_maps = shard_inputs(inputs, cfg)
    res = run_bass_kernel_spmd(nc, in_maps, core_ids=list(range(8)))
    return gather_outputs(res.results, cfg, cfg.H, cfg.W)
